# revision 30
# baseline (speedup 1.0000x reference)
"""GAT block (graph attention) Bass/Tile kernel for Trainium2, 8 NeuronCores.

Full-input contract: kernel(x=(8,2048,128), W=(128,64), a=(128,1)) -> (8,2048,64).
Sharding: data-parallel over batch - one batch element per core, W/a replicated,
zero inter-core communication; host stacks (and un-transposes) per-core outputs.

Per-core math (N=2048, Fin=128, Fout=64):
  h  = x @ W                               (N, Fout)
  s1 = h @ a[:64, 0],  s2 = h @ a[64:, 0]  (N,)
  e[i, j] = leakyrelu(s1[i] + s2[j], 0.2)
  att     = softmax(e, axis=0)   (normalize over i for each column j)
  out     = leakyrelu(att @ h, 0.2)

Key trick: because exp is monotone,
  exp(lrelu(z)) = max(exp(z), exp(z/5)),
so with eA=exp(s1), eC=exp(s1/5) broadcast along partitions and per-tile
columns eB=exp(s2), eD=exp(s2/5):
  Pt[j, i] = exp(lrelu(s1[i]+s2[j])) = max(eB[j]*eA[i], eD[j]*eC[i]).
This makes each (128, 2048) attention tile a single fused DVE op
(max(Src0*C0, Src1*C1) with a free row-sum accumulator for the softmax
denominator), so the two elementwise engines split the 16 tiles 11/5:
  * DVE (11 tiles): the fused custom op, one ~2.4us pass per tile.
  * ACT (5 tiles): Prelu (per-partition bias = s2 col, reading the s1
    broadcast from PSUM) then Exp with accum_out - 2 passes, ~3.95us.
  (gpsimd is useless here: no PSUM access, ~2us dispatch per op,
   2.6 cyc/elem.)
Other structure:
  * s1 broadcast (s1b) is built by a single fused matmul per 512-chunk:
    lhsT = Q1 (Q1[k,p] = (W a1)[k] for all p) against xT chunks, directly
    into a persistent 4-bank PSUM tile (no SBUF copy; ACT reads PSUM).
  * eA/eC broadcast matrices come from full-pass ACT exp over s1b (bf16 out).
  * x is DMAed in 4 batched transfers (4 row-tiles per descriptor set),
    transposed on PE; h tiles (with s1/s2 score columns appended to W)
    come from one f32r matmul per tile.
  * The output is accumulated transposed (hpT[f,i]) in 4 PSUM banks by 64
    bf16 matmuls; final leakyrelu runs from PSUM; host un-transposes.
"""

import numpy as np
from contextlib import ExitStack
from operator import add as _op_add

import concourse.bass as bass
import concourse.mybir as mybir
import concourse.tile as tile
from concourse import bacc
from concourse._compat import with_exitstack
from concourse.bass_utils import run_bass_kernel_spmd
from concourse.masks import make_identity

# ---- custom DVE op: out = max(in0*s0, in1*s1), accum_out = rowsum(out) ----
import numpy as _np
from concourse import dve_ops as _dvo
from concourse.dve_spec import (
    Spec as _Spec, Src0 as _Src0, Src1 as _Src1, C0 as _C0, C1 as _C1, C2 as _C2,
    Zero as _Zero, maxx as _maxx, lower as _dve_lower,
    _has_src1 as _dve_has_src1,
)
from concourse.dve_uop import DveOpSpec as _DveOpSpec


def _register_maxmul():
    name = "MAXMUL_GAT_ANT"
    if name in _dvo._SUB_OPCODE_FOR_NAME:
        return next(o for o in _dvo.OPS if o.name == name)

    def _ref(in0, in1, s0, s1, imm2):
        b = _np.maximum(
            in0.astype(_np.float32) * s0, in1.astype(_np.float32) * s1
        ).astype(_np.float32)
        return b, b.reshape(b.shape[0], -1).sum(axis=-1, keepdims=True)

    spec = _Spec(body=_maxx(_Src0 * _C0, _Src1 * _C1),
                 accum=_op_add, accum_init=_Zero, reference=_ref)
    op = _dvo.DveOp(name, spec, subdim=False, uops_sha={},
                    perf_en={"v3": True, "v4": True})
    row = _dvo._CUSTOM_DVE_ROW_BASE + len(_dvo.OPS)
    assert row < 0x20
    _dvo.OPS.append(op)
    _dvo.CUSTOM_DVE_SPECS[name] = spec
    _dvo._SUB_OPCODE_FOR_NAME[name] = row
    for ver in ("v3", "v4"):
        try:
            s = _DveOpSpec(name=name, opcode=row, uops=_dve_lower(spec, ver=ver),
                           rd1_en=_dve_has_src1(spec)).sha(ver)
            op.uops_sha[ver] = s
        except Exception:
            pass
    return op


_MAXMUL = _register_maxmul()


def _register_lrelu1():
    name = "LRELU1_GAT_ANT"
    if name in _dvo._SUB_OPCODE_FOR_NAME:
        return next(o for o in _dvo.OPS if o.name == name)

    def _ref(in0, in1, s0, s1, imm2):
        v = in0.astype(_np.float32)
        return _np.maximum(v * imm2, v).astype(_np.float32)

    spec = _Spec(body=_maxx(_Src0 * _C2, _Src0), reference=_ref)
    op = _dvo.DveOp(name, spec, subdim=False, uops_sha={},
                    perf_en={"v3": True, "v4": True})
    row = _dvo._CUSTOM_DVE_ROW_BASE + len(_dvo.OPS)
    assert row < 0x20
    _dvo.OPS.append(op)
    _dvo.CUSTOM_DVE_SPECS[name] = spec
    _dvo._SUB_OPCODE_FOR_NAME[name] = row
    for ver in ("v3", "v4"):
        try:
            sh = _DveOpSpec(name=name, opcode=row, uops=_dve_lower(spec, ver=ver),
                            rd1_en=_dve_has_src1(spec)).sha(ver)
            op.uops_sha[ver] = sh
        except Exception:
            pass
    return op


_LRELU1 = _register_lrelu1()

F32 = mybir.dt.float32
F32R = mybir.dt.float32r
BF16 = mybir.dt.bfloat16
AF = mybir.ActivationFunctionType
ALU = mybir.AluOpType

N = 2048
FIN = 128
FOUT = 64
P = 128
T = N // P          # 16 row tiles
NCH = N // 512      # 4 chunks
NEG_SLOPE = 0.2
N_CORES = 8

# engine per attention j-tile: A=ACT (prelu+exp), D=DVE (fused max-mul),
# P=Pool (2 stock passes)
# gpsimd (Pool) cannot access PSUM, has ~2us fixed dispatch and ~2.6
# cyc/elem, so attention tiles go on DVE (1 fused pass, ~2.4us) and ACT
# (prelu+exp, ~3.95us) only.  4/12 split balances the engines once DVE
# carries the xT casts and ACT the h copies / h scaling.
ENG = ['A', 'D', 'D', 'A', 'D', 'D', 'A', 'D',
       'D', 'A', 'D', 'D', 'A', 'D', 'D', 'D']


@with_exitstack
def _gat_body(ctx: ExitStack, tc: tile.TileContext, x, w, a, out):
    nc = tc.nc

    const = ctx.enter_context(tc.tile_pool(name="const", bufs=1))
    xin = ctx.enter_context(tc.tile_pool(name="xin", bufs=4))
    ascr = ctx.enter_context(tc.tile_pool(name="ascr", bufs=2))
    pscr = ctx.enter_context(tc.tile_pool(name="pscr", bufs=2))

    # ---- persistent SBUF tiles ----
    ident = const.tile([P, P], F32)
    make_identity(nc, ident)
    # host precomputes wsa = [W | W@a1 | W@a2] and the (W@a1) row, killing
    # the serial wT->wa->waT->q1 cross-engine prep chain
    wsa_raw = const.tile([FIN, FOUT + 2], F32)
    nc.sync.dma_start(wsa_raw[:], w)
    warow_raw = const.tile([1, P], F32)
    nc.gpsimd.dma_start(warow_raw[:], a)
    warow = const.tile([1, P], F32R)
    nc.vector.tensor_copy(warow[:], warow_raw[:])
    ones_raw = const.tile([1, P], F32)
    nc.vector.memset(ones_raw[:], 1.0)
    ones_row = const.tile([1, P], F32R)
    nc.vector.tensor_copy(ones_row[:], ones_raw[:])

    xT = const.tile([P, T, P], F32R)          # x transposed: [k, t, n]
    hs12 = const.tile([P, T, FOUT + 2], F32)  # [h | s1 s2 cols] per tile
    hs_bf = const.tile([P, T, FOUT], BF16)    # h/denom in bf16
    wsa = const.tile([FIN, FOUT + 2], F32R)   # [W | W@a1 | W@a2]
    eab = const.tile([P, N], BF16)            # exp(s1) bcast along partitions
    ecb = const.tile([P, N], BF16)            # exp(s1/5) bcast
    ebd = const.tile([P, T, 2], F32)          # per tile [exp(s2), exp(s2/5)]
    p_all = const.tile([P, T, N], BF16)       # attention numerator, transposed
    o_sb = const.tile([FOUT, N], F32)         # output transposed
    dens = const.tile([P, T], F32)
    rden = const.tile([P, T], F32)

    # s1 broadcast lives in PSUM (4 banks), read directly by ACT
    s1b_pool = ctx.enter_context(tc.tile_pool(name="s1b", bufs=1, space="PSUM"))
    s1b = s1b_pool.tile([P, N], F32)

    _scr0 = [None]

    def emit_tile(t):
        e = ENG[t]
        s2c = hs12[:, t, FOUT + 1:FOUT + 2]
        if e == 'A':
            if t == 0 and _scr0[0] is not None:
                scr = _scr0[0]
            else:
                scr = ascr.tile([P, N], F32, tag="as", name=f"as{t}")
                nc.scalar.activation(scr[:], s1b[:], AF.Prelu, bias=s2c,
                                     scale=1.0, alpha=NEG_SLOPE)
            nc.scalar.activation(p_all[:, t, :], scr[:], AF.Exp,
                                 accum_out=dens[:, t:t + 1])
        else:
            nc.vector._custom_dve(_MAXMUL, out=p_all[:, t, :],
                                  accum_out=dens[:, t:t + 1],
                                  in0=eab[:], in1=ecb[:],
                                  s0=ebd[:, t, 0:1], s1=ebd[:, t, 1:2])

    with tc.tile_pool(name="ps_m", bufs=2, space="PSUM") as ps_m, \
         tc.tile_pool(name="ps_tr", bufs=2, space="PSUM") as ps_tr:
        # short junk bf16 burst: ramps the PE p-state while DMAs land
        wup = const.tile([P, 512], BF16)
        nc.vector.memset(wup[:], 0.0)
        for i in range(2):
            ps_w = ps_m.tile([P, 512], F32, tag="m", name=f"wup{i}")
            nc.tensor.matmul(ps_w[:], lhsT=wup[:, 0:P], rhs=wup[:],
                             start=True, stop=True)

        # wsa cast + Q1[k, p] = (W a1)[k] for all p (K=1 broadcast)
        nc.vector.tensor_copy(wsa[:], wsa_raw[:])
        ps_q1 = ps_m.tile([P, P], F32, tag="m", name="q1")
        nc.tensor.matmul(ps_q1[:], lhsT=warow[:], rhs=ones_row[:],
                         start=True, stop=True)
        q1 = const.tile([P, P], F32R)
        nc.vector.tensor_copy(q1[:], ps_q1[:])

        # x DMAs: one per row-tile (finer completion granularity lets each
        # chunk's transposes start as soon as its own 64KB lands)
        xg = [xin.tile([P, 4, P], F32, tag="xg", name=f"xg{g}") for g in range(4)]
        x_engs = [nc.sync, nc.gpsimd, nc.scalar]
        for t in range(T):
            g, ci = t // 4, t % 4
            x_engs[t % 3].dma_start(xg[g][:, ci, :], x[t * P:(t + 1) * P, :])

        # Score path first and alone on the PE queue: per chunk, 4
        # transposes (casts on DVE, which is idle in the prologue) then the
        # fused s1b broadcast matmul straight into PSUM; eA/eC exps (ACT)
        # read s1b from PSUM in 1024-wide pieces after odd chunks.  The h
        # matmuls for ALL chunks are emitted after, so their PSUM-buffer
        # serialization never blocks the next chunk's transposes.
        for c in range(NCH):
            psT = ps_tr.tile([P, 4, P], F32, tag="tr", name=f"trc{c}")
            for ci in range(4):
                nc.tensor.transpose(psT[:, ci, :], xg[c][:, ci, :], ident[:])
            nc.vector.tensor_copy(xT[:, 4 * c:4 * c + 4, :], psT[:])
            sl = slice(c * 512, (c + 1) * 512)
            nc.tensor.matmul(s1b[:, sl], lhsT=q1[:],
                             rhs=xT[:, 4 * c:4 * c + 4, :],
                             start=True, stop=True)
            # junk matmuls keep the PE queue dense so the p-state ramps
            for i in range(2):
                ps_w = ps_m.tile([P, 512], F32, tag="m", name=f"wupc{c}_{i}")
                nc.tensor.matmul(ps_w[:], lhsT=wup[:, 0:P], rhs=wup[:],
                                 start=True, stop=True)
            nc.scalar.activation(eab[:, sl], s1b[:, sl], AF.Exp)
            nc.scalar.activation(ecb[:, sl], s1b[:, sl], AF.Exp, scale=0.2)
        def h_tile(t):
            psh = ps_m.tile([P, FOUT + 2], F32, tag="m", name=f"h{t}")
            nc.tensor.matmul(psh[:], lhsT=xT[:, t, :], rhs=wsa[:],
                             start=True, stop=True)
            if t % 2 == 0:
                nc.scalar.copy(hs12[:, t, :], psh[:])
            else:
                nc.vector.tensor_copy(hs12[:, t, :], psh[:])
            if t % 4 == 3:
                s2g = hs12[:, t - 3:t + 1, FOUT + 1:FOUT + 2]
                nc.scalar.activation(ebd[:, t - 3:t + 1, 0:1], s2g, AF.Exp)
                nc.scalar.activation(ebd[:, t - 3:t + 1, 1:2], s2g,
                                     AF.Exp, scale=0.2)

        # chunk-0 h tiles first, then kick off the first two DVE stream
        # tiles early (they need only eab/ecb + chunk-0 ebd), then the rest
        for t in range(4):
            h_tile(t)
        emit_tile(1)
        emit_tile(2)
        for t in range(4, T):
            h_tile(t)

    # setup PSUM pools released; output accumulators take those banks
    ps_out = ctx.enter_context(tc.tile_pool(name="ps_out", bufs=1, space="PSUM"))
    hp = [ps_out.tile([FOUT, 512], F32, tag=f"hp{c}", name=f"hp{c}")
          for c in range(NCH)]

    # ---- main stream: one P-tile op (or pair) per tile on its engine,
    # then recip -> hbf scale -> 4 output matmuls ----
    n_done = [0]


    def emit_post(t):
        k = n_done[0]
        if k % 2 == 1:
            # one paired reciprocal covers tiles t-1 and t
            nc.vector.reciprocal(rden[:, t - 1:t + 1], dens[:, t - 1:t + 1])
        elif k == T - 1:
            nc.vector.reciprocal(rden[:, t:t + 1], dens[:, t:t + 1])
        for u in ([t - 1, t] if k % 2 == 1 else ([t] if k == T - 1 else [])):
            if u % 2 == 0:
                nc.scalar.activation(hs_bf[:, u, :], hs12[:, u, 0:FOUT],
                                     AF.Copy, scale=rden[:, u:u + 1])
            else:
                nc.vector.tensor_scalar_mul(hs_bf[:, u, :],
                                            hs12[:, u, 0:FOUT],
                                            rden[:, u:u + 1])
            for c in range(NCH):
                nc.tensor.matmul(hp[c][:], lhsT=hs_bf[:, u, :],
                                 rhs=p_all[:, u, c * 512:(c + 1) * 512],
                                 start=(u == 0), stop=(u == T - 1))
        n_done[0] += 1

    for t in range(T):
        if t not in (1, 2):
            emit_tile(t)
        if t > 0:
            emit_post(t - 1)
    emit_post(T - 1)

    # ---- epilogue: leakyrelu straight from PSUM, DMA out transposed ----
    out_engs = [nc.sync, nc.gpsimd, nc.sync, nc.gpsimd]
    for c in range(NCH):
        sl = slice(c * 512, (c + 1) * 512)
        if c % 2 == 0:
            nc.scalar.activation(o_sb[:, sl], hp[c][:], AF.Prelu,
                                 bias=0.0, scale=1.0, alpha=NEG_SLOPE)
        else:
            nc.vector._custom_dve(_LRELU1, out=o_sb[:, sl], in0=hp[c][:],
                                  imm2=NEG_SLOPE)
        # split each chunk's store across two queues; 22.5 GB/s per queue
        h1 = slice(c * 512, c * 512 + 256)
        h2 = slice(c * 512 + 256, (c + 1) * 512)
        out_engs[c].dma_start(out[:, h1], o_sb[:, h1])
        out_engs[(c + 1) % 4].dma_start(out[:, h2], o_sb[:, h2])


_NC_CACHE = {}


def _build_nc():
    if "nc" in _NC_CACHE:
        return _NC_CACHE["nc"]
    nc = bacc.Bacc("TRN2", target_bir_lowering=False, debug=False)
    x = nc.dram_tensor("x", (N, FIN), F32, kind="ExternalInput").ap()
    w = nc.dram_tensor("w", (FIN, FOUT + 2), F32, kind="ExternalInput").ap()
    a = nc.dram_tensor("a", (1, P), F32, kind="ExternalInput").ap()
    # transposed output; the host un-transposes
    out = nc.dram_tensor("out", (FOUT, N), F32, kind="ExternalOutput").ap()
    with tile.TileContext(nc) as tc:
        _gat_body(tc, x, w, a, out)
    nc.compile()
    _NC_CACHE["nc"] = nc
    return nc


def host_prep(W, a):
    # tiny host-side prep: wa = W @ [a1, a2]; wsa = [W | wa]; q1 row = wa1^T
    W = np.ascontiguousarray(np.asarray(W), dtype=np.float32)
    a = np.ascontiguousarray(np.asarray(a), dtype=np.float32)
    wa = W @ np.stack([a[:FOUT, 0], a[FOUT:, 0]], axis=1)
    wsa_host = np.ascontiguousarray(
        np.concatenate([W, wa], axis=1), dtype=np.float32)
    warow_host = np.ascontiguousarray(wa[:, 0].reshape(1, P), dtype=np.float32)
    return wsa_host, warow_host


def kernel(x, W, a):
    x = np.ascontiguousarray(np.asarray(x), dtype=np.float32)
    assert x.shape == (N_CORES, N, FIN), x.shape
    nc = _build_nc()
    wsa_host, warow_host = host_prep(W, a)
    in_maps = [{"x": x[c], "w": wsa_host, "a": warow_host}
               for c in range(N_CORES)]
    res = run_bass_kernel_spmd(nc, in_maps, core_ids=list(range(N_CORES)))
    return np.stack([res.results[c]["out"].T.copy() for c in range(N_CORES)], axis=0)


# revision 31
# speedup vs baseline: 1.0006x; 1.0006x over previous
"""GAT block (graph attention) Bass/Tile kernel for Trainium2, 8 NeuronCores.

Full-input contract: kernel(x=(8,2048,128), W=(128,64), a=(128,1)) -> (8,2048,64).
Sharding: data-parallel over batch - one batch element per core, W/a replicated,
zero inter-core communication; host stacks (and un-transposes) per-core outputs.

Per-core math (N=2048, Fin=128, Fout=64):
  h  = x @ W                               (N, Fout)
  s1 = h @ a[:64, 0],  s2 = h @ a[64:, 0]  (N,)
  e[i, j] = leakyrelu(s1[i] + s2[j], 0.2)
  att     = softmax(e, axis=0)   (normalize over i for each column j)
  out     = leakyrelu(att @ h, 0.2)

Key trick: because exp is monotone,
  exp(lrelu(z)) = max(exp(z), exp(z/5)),
so with eA=exp(s1), eC=exp(s1/5) broadcast along partitions and per-tile
columns eB=exp(s2), eD=exp(s2/5):
  Pt[j, i] = exp(lrelu(s1[i]+s2[j])) = max(eB[j]*eA[i], eD[j]*eC[i]).
This makes each (128, 2048) attention tile a single fused DVE op
(max(Src0*C0, Src1*C1) with a free row-sum accumulator for the softmax
denominator), so the two elementwise engines split the 16 tiles 11/5:
  * DVE (11 tiles): the fused custom op, one ~2.4us pass per tile.
  * ACT (5 tiles): Prelu (per-partition bias = s2 col, reading the s1
    broadcast from PSUM) then Exp with accum_out - 2 passes, ~3.95us.
  (gpsimd is useless here: no PSUM access, ~2us dispatch per op,
   2.6 cyc/elem.)
Other structure:
  * s1 broadcast (s1b) is built by a single fused matmul per 512-chunk:
    lhsT = Q1 (Q1[k,p] = (W a1)[k] for all p) against xT chunks, directly
    into a persistent 4-bank PSUM tile (no SBUF copy; ACT reads PSUM).
  * eA/eC broadcast matrices come from full-pass ACT exp over s1b (bf16 out).
  * x is DMAed in 4 batched transfers (4 row-tiles per descriptor set),
    transposed on PE; h tiles (with s1/s2 score columns appended to W)
    come from one f32r matmul per tile.
  * The output is accumulated transposed (hpT[f,i]) in 4 PSUM banks by 64
    bf16 matmuls; final leakyrelu runs from PSUM; host un-transposes.
"""

import numpy as np
from contextlib import ExitStack
from operator import add as _op_add

import concourse.bass as bass
import concourse.mybir as mybir
import concourse.tile as tile
from concourse import bacc
from concourse._compat import with_exitstack
from concourse.bass_utils import run_bass_kernel_spmd
from concourse.masks import make_identity

# ---- custom DVE op: out = max(in0*s0, in1*s1), accum_out = rowsum(out) ----
import numpy as _np
from concourse import dve_ops as _dvo
from concourse.dve_spec import (
    Spec as _Spec, Src0 as _Src0, Src1 as _Src1, C0 as _C0, C1 as _C1, C2 as _C2,
    Zero as _Zero, maxx as _maxx, lower as _dve_lower,
    _has_src1 as _dve_has_src1,
)
from concourse.dve_uop import DveOpSpec as _DveOpSpec


def _register_maxmul():
    name = "MAXMUL_GAT_ANT"
    if name in _dvo._SUB_OPCODE_FOR_NAME:
        return next(o for o in _dvo.OPS if o.name == name)

    def _ref(in0, in1, s0, s1, imm2):
        b = _np.maximum(
            in0.astype(_np.float32) * s0, in1.astype(_np.float32) * s1
        ).astype(_np.float32)
        return b, b.reshape(b.shape[0], -1).sum(axis=-1, keepdims=True)

    spec = _Spec(body=_maxx(_Src0 * _C0, _Src1 * _C1),
                 accum=_op_add, accum_init=_Zero, reference=_ref)
    op = _dvo.DveOp(name, spec, subdim=False, uops_sha={},
                    perf_en={"v3": True, "v4": True})
    row = _dvo._CUSTOM_DVE_ROW_BASE + len(_dvo.OPS)
    assert row < 0x20
    _dvo.OPS.append(op)
    _dvo.CUSTOM_DVE_SPECS[name] = spec
    _dvo._SUB_OPCODE_FOR_NAME[name] = row
    for ver in ("v3", "v4"):
        try:
            s = _DveOpSpec(name=name, opcode=row, uops=_dve_lower(spec, ver=ver),
                           rd1_en=_dve_has_src1(spec)).sha(ver)
            op.uops_sha[ver] = s
        except Exception:
            pass
    return op


_MAXMUL = _register_maxmul()


def _register_lrelu1():
    name = "LRELU1_GAT_ANT"
    if name in _dvo._SUB_OPCODE_FOR_NAME:
        return next(o for o in _dvo.OPS if o.name == name)

    def _ref(in0, in1, s0, s1, imm2):
        v = in0.astype(_np.float32)
        return _np.maximum(v * imm2, v).astype(_np.float32)

    spec = _Spec(body=_maxx(_Src0 * _C2, _Src0), reference=_ref)
    op = _dvo.DveOp(name, spec, subdim=False, uops_sha={},
                    perf_en={"v3": True, "v4": True})
    row = _dvo._CUSTOM_DVE_ROW_BASE + len(_dvo.OPS)
    assert row < 0x20
    _dvo.OPS.append(op)
    _dvo.CUSTOM_DVE_SPECS[name] = spec
    _dvo._SUB_OPCODE_FOR_NAME[name] = row
    for ver in ("v3", "v4"):
        try:
            sh = _DveOpSpec(name=name, opcode=row, uops=_dve_lower(spec, ver=ver),
                            rd1_en=_dve_has_src1(spec)).sha(ver)
            op.uops_sha[ver] = sh
        except Exception:
            pass
    return op


_LRELU1 = _register_lrelu1()

F32 = mybir.dt.float32
F32R = mybir.dt.float32r
BF16 = mybir.dt.bfloat16
AF = mybir.ActivationFunctionType
ALU = mybir.AluOpType

N = 2048
FIN = 128
FOUT = 64
P = 128
T = N // P          # 16 row tiles
NCH = N // 512      # 4 chunks
NEG_SLOPE = 0.2
N_CORES = 8

# engine per attention j-tile: A=ACT (prelu+exp), D=DVE (fused max-mul),
# P=Pool (2 stock passes)
# gpsimd (Pool) cannot access PSUM, has ~2us fixed dispatch and ~2.6
# cyc/elem, so attention tiles go on DVE (1 fused pass, ~2.4us) and ACT
# (prelu+exp, ~3.95us) only.  4/12 split balances the engines once DVE
# carries the xT casts and ACT the h copies / h scaling.
ENG = ['A', 'D', 'D', 'A', 'D', 'D', 'A', 'D',
       'D', 'A', 'D', 'D', 'A', 'D', 'D', 'D']


@with_exitstack
def _gat_body(ctx: ExitStack, tc: tile.TileContext, x, w, a, out):
    nc = tc.nc

    const = ctx.enter_context(tc.tile_pool(name="const", bufs=1))
    xin = ctx.enter_context(tc.tile_pool(name="xin", bufs=4))
    ascr = ctx.enter_context(tc.tile_pool(name="ascr", bufs=2))
    pscr = ctx.enter_context(tc.tile_pool(name="pscr", bufs=2))

    # ---- persistent SBUF tiles ----
    ident = const.tile([P, P], F32)
    make_identity(nc, ident)
    # host precomputes wsa = [W | W@a1 | W@a2] and the (W@a1) row, killing
    # the serial wT->wa->waT->q1 cross-engine prep chain
    wsa_raw = const.tile([FIN, FOUT + 2], F32)
    nc.sync.dma_start(wsa_raw[:], w)
    warow_raw = const.tile([1, P], F32)
    nc.gpsimd.dma_start(warow_raw[:], a)
    warow = const.tile([1, P], F32R)
    nc.vector.tensor_copy(warow[:], warow_raw[:])
    ones_raw = const.tile([1, P], F32)
    nc.vector.memset(ones_raw[:], 1.0)
    ones_row = const.tile([1, P], F32R)
    nc.vector.tensor_copy(ones_row[:], ones_raw[:])

    xT = const.tile([P, T, P], F32R)          # x transposed: [k, t, n]
    hs12 = const.tile([P, T, FOUT + 2], F32)  # [h | s1 s2 cols] per tile
    hs_bf = const.tile([P, T, FOUT], BF16)    # h/denom in bf16
    wsa = const.tile([FIN, FOUT + 2], F32R)   # [W | W@a1 | W@a2]
    eab = const.tile([P, N], BF16)            # exp(s1) bcast along partitions
    ecb = const.tile([P, N], BF16)            # exp(s1/5) bcast
    ebd = const.tile([P, T, 2], F32)          # per tile [exp(s2), exp(s2/5)]
    p_all = const.tile([P, T, N], BF16)       # attention numerator, transposed
    o_sb = const.tile([FOUT, N], F32)         # output transposed
    dens = const.tile([P, T], F32)
    rden = const.tile([P, T], F32)

    # s1 broadcast lives in PSUM (4 banks), read directly by ACT
    s1b_pool = ctx.enter_context(tc.tile_pool(name="s1b", bufs=1, space="PSUM"))
    s1b = s1b_pool.tile([P, N], F32)

    def emit_tile(t):
        e = ENG[t]
        s2c = hs12[:, t, FOUT + 1:FOUT + 2]
        if e == 'A':
            scr = ascr.tile([P, N], F32, tag="as", name=f"as{t}")
            nc.scalar.activation(scr[:], s1b[:], AF.Prelu, bias=s2c,
                                 scale=1.0, alpha=NEG_SLOPE)
            nc.scalar.activation(p_all[:, t, :], scr[:], AF.Exp,
                                 accum_out=dens[:, t:t + 1])
        else:
            nc.vector._custom_dve(_MAXMUL, out=p_all[:, t, :],
                                  accum_out=dens[:, t:t + 1],
                                  in0=eab[:], in1=ecb[:],
                                  s0=ebd[:, t, 0:1], s1=ebd[:, t, 1:2])

    with tc.tile_pool(name="ps_m", bufs=2, space="PSUM") as ps_m, \
         tc.tile_pool(name="ps_tr", bufs=2, space="PSUM") as ps_tr:
        # short junk bf16 burst: ramps the PE p-state while DMAs land
        wup = const.tile([P, 512], BF16)
        nc.vector.memset(wup[:], 0.0)
        for i in range(2):
            ps_w = ps_m.tile([P, 512], F32, tag="m", name=f"wup{i}")
            nc.tensor.matmul(ps_w[:], lhsT=wup[:, 0:P], rhs=wup[:],
                             start=True, stop=True)

        # wsa cast + Q1[k, p] = (W a1)[k] for all p (K=1 broadcast)
        nc.vector.tensor_copy(wsa[:], wsa_raw[:])
        ps_q1 = ps_m.tile([P, P], F32, tag="m", name="q1")
        nc.tensor.matmul(ps_q1[:], lhsT=warow[:], rhs=ones_row[:],
                         start=True, stop=True)
        q1 = const.tile([P, P], F32R)
        nc.vector.tensor_copy(q1[:], ps_q1[:])

        # x DMAs: one per row-tile (finer completion granularity lets each
        # chunk's transposes start as soon as its own 64KB lands)
        xg = [xin.tile([P, 4, P], F32, tag="xg", name=f"xg{g}") for g in range(4)]
        x_engs = [nc.sync, nc.gpsimd, nc.scalar]
        for t in range(T):
            g, ci = t // 4, t % 4
            x_engs[t % 3].dma_start(xg[g][:, ci, :], x[t * P:(t + 1) * P, :])

        # Score path first and alone on the PE queue: per chunk, 4
        # transposes (casts on DVE, which is idle in the prologue) then the
        # fused s1b broadcast matmul straight into PSUM; eA/eC exps (ACT)
        # read s1b from PSUM in 1024-wide pieces after odd chunks.  The h
        # matmuls for ALL chunks are emitted after, so their PSUM-buffer
        # serialization never blocks the next chunk's transposes.
        for c in range(NCH):
            psT = ps_tr.tile([P, 4, P], F32, tag="tr", name=f"trc{c}")
            for ci in range(4):
                nc.tensor.transpose(psT[:, ci, :], xg[c][:, ci, :], ident[:])
            nc.vector.tensor_copy(xT[:, 4 * c:4 * c + 4, :], psT[:])
            sl = slice(c * 512, (c + 1) * 512)
            nc.tensor.matmul(s1b[:, sl], lhsT=q1[:],
                             rhs=xT[:, 4 * c:4 * c + 4, :],
                             start=True, stop=True)
            nc.scalar.activation(eab[:, sl], s1b[:, sl], AF.Exp)
            nc.scalar.activation(ecb[:, sl], s1b[:, sl], AF.Exp, scale=0.2)
        def h_tile(t):
            psh = ps_m.tile([P, FOUT + 2], F32, tag="m", name=f"h{t}")
            nc.tensor.matmul(psh[:], lhsT=xT[:, t, :], rhs=wsa[:],
                             start=True, stop=True)
            if t % 2 == 0:
                nc.scalar.copy(hs12[:, t, :], psh[:])
            else:
                nc.vector.tensor_copy(hs12[:, t, :], psh[:])
            if t % 4 == 3:
                s2g = hs12[:, t - 3:t + 1, FOUT + 1:FOUT + 2]
                nc.scalar.activation(ebd[:, t - 3:t + 1, 0:1], s2g, AF.Exp)
                nc.scalar.activation(ebd[:, t - 3:t + 1, 1:2], s2g,
                                     AF.Exp, scale=0.2)

        # chunk-0 h tiles first, then kick off the first two DVE stream
        # tiles early (they need only eab/ecb + chunk-0 ebd), then the rest
        for t in range(4):
            h_tile(t)
        emit_tile(1)
        emit_tile(2)
        for t in range(4, T):
            h_tile(t)

    # setup PSUM pools released; output accumulators take those banks
    ps_out = ctx.enter_context(tc.tile_pool(name="ps_out", bufs=1, space="PSUM"))
    hp = [ps_out.tile([FOUT, 512], F32, tag=f"hp{c}", name=f"hp{c}")
          for c in range(NCH)]

    # ---- main stream: one P-tile op (or pair) per tile on its engine,
    # then recip -> hbf scale -> 4 output matmuls ----
    n_done = [0]


    def emit_post(t):
        k = n_done[0]
        if k % 2 == 1:
            # one paired reciprocal covers tiles t-1 and t
            nc.vector.reciprocal(rden[:, t - 1:t + 1], dens[:, t - 1:t + 1])
        elif k == T - 1:
            nc.vector.reciprocal(rden[:, t:t + 1], dens[:, t:t + 1])
        for u in ([t - 1, t] if k % 2 == 1 else ([t] if k == T - 1 else [])):
            if u % 2 == 0:
                nc.scalar.activation(hs_bf[:, u, :], hs12[:, u, 0:FOUT],
                                     AF.Copy, scale=rden[:, u:u + 1])
            else:
                nc.vector.tensor_scalar_mul(hs_bf[:, u, :],
                                            hs12[:, u, 0:FOUT],
                                            rden[:, u:u + 1])
            for c in range(NCH):
                nc.tensor.matmul(hp[c][:], lhsT=hs_bf[:, u, :],
                                 rhs=p_all[:, u, c * 512:(c + 1) * 512],
                                 start=(u == 0), stop=(u == T - 1))
        n_done[0] += 1

    for t in range(T):
        if t not in (1, 2):
            emit_tile(t)
        if t > 0:
            emit_post(t - 1)
    emit_post(T - 1)

    # ---- epilogue: leakyrelu straight from PSUM, DMA out transposed ----
    out_engs = [nc.sync, nc.gpsimd, nc.sync, nc.gpsimd]
    for c in range(NCH):
        sl = slice(c * 512, (c + 1) * 512)
        if c % 2 == 0:
            nc.scalar.activation(o_sb[:, sl], hp[c][:], AF.Prelu,
                                 bias=0.0, scale=1.0, alpha=NEG_SLOPE)
        else:
            nc.vector._custom_dve(_LRELU1, out=o_sb[:, sl], in0=hp[c][:],
                                  imm2=NEG_SLOPE)
        # split each chunk's store across two queues; 22.5 GB/s per queue
        h1 = slice(c * 512, c * 512 + 256)
        h2 = slice(c * 512 + 256, (c + 1) * 512)
        out_engs[c].dma_start(out[:, h1], o_sb[:, h1])
        out_engs[(c + 1) % 4].dma_start(out[:, h2], o_sb[:, h2])


_NC_CACHE = {}


def _build_nc():
    if "nc" in _NC_CACHE:
        return _NC_CACHE["nc"]
    nc = bacc.Bacc("TRN2", target_bir_lowering=False, debug=False)
    x = nc.dram_tensor("x", (N, FIN), F32, kind="ExternalInput").ap()
    w = nc.dram_tensor("w", (FIN, FOUT + 2), F32, kind="ExternalInput").ap()
    a = nc.dram_tensor("a", (1, P), F32, kind="ExternalInput").ap()
    # transposed output; the host un-transposes
    out = nc.dram_tensor("out", (FOUT, N), F32, kind="ExternalOutput").ap()
    with tile.TileContext(nc) as tc:
        _gat_body(tc, x, w, a, out)
    nc.compile()
    _NC_CACHE["nc"] = nc
    return nc


def host_prep(W, a):
    # tiny host-side prep: wa = W @ [a1, a2]; wsa = [W | wa]; q1 row = wa1^T
    W = np.ascontiguousarray(np.asarray(W), dtype=np.float32)
    a = np.ascontiguousarray(np.asarray(a), dtype=np.float32)
    wa = W @ np.stack([a[:FOUT, 0], a[FOUT:, 0]], axis=1)
    wsa_host = np.ascontiguousarray(
        np.concatenate([W, wa], axis=1), dtype=np.float32)
    warow_host = np.ascontiguousarray(wa[:, 0].reshape(1, P), dtype=np.float32)
    return wsa_host, warow_host


def kernel(x, W, a):
    x = np.ascontiguousarray(np.asarray(x), dtype=np.float32)
    assert x.shape == (N_CORES, N, FIN), x.shape
    nc = _build_nc()
    wsa_host, warow_host = host_prep(W, a)
    in_maps = [{"x": x[c], "w": wsa_host, "a": warow_host}
               for c in range(N_CORES)]
    res = run_bass_kernel_spmd(nc, in_maps, core_ids=list(range(N_CORES)))
    return np.stack([res.results[c]["out"].T.copy() for c in range(N_CORES)], axis=0)
